# revision 3
# baseline (speedup 1.0000x reference)
"""BitLinear (BitNet 1.58-bit ternary) distributed Trainium2 kernel.

Reference semantics:
    scale = max(mean(|w|), 1e-5)
    w_q   = sign(w) * (|w| > scale/3)          # ternary {-1, 0, 1}
    out   = (x @ w_q.T) * scale                # x: [4, 2048, 2048], w: [2048, 2048]

Sharding: data-parallel over tokens. The 8192 tokens are split 1024/core;
the weight is replicated so each core computes the scale locally (no
collectives). Host-side prep transposes both operands so the contraction
dim (in_features) lands on SBUF partitions, and pre-casts x to bf16.

Device math: the ternary quantization is computed as
    wq2 = Sign(w + t) + Sign(w - t)   with t = scale/3
which is {-2, 0, 2} (exactly representable in bf16); the missing factor
1/2 is folded into the output scaling (psum * scale/2).
"""

import sys

sys.path.insert(0, "/opt/trn_rl_repo")

import numpy as np

N_CORES = 8
B, S, D = 4, 2048, 2048        # x: [B, S, D]
OUT = 2048                     # out_features
TOK = B * S                    # 8192 tokens
TPC = TOK // N_CORES           # 1024 tokens per core
KT = D // 128                  # 16 K-tiles of 128
MT = TPC // 128                # 8 M-tiles per core
NT = OUT // 512                # 4 N-tiles of 512
N_ELEM = float(D * OUT)        # elements of w
EPS = 1e-5


def build_kernel():
    from concourse import bacc, tile, mybir

    f32 = mybir.dt.float32
    bf16 = mybir.dt.bfloat16
    Alu = mybir.AluOpType
    Act = mybir.ActivationFunctionType

    nc = bacc.Bacc(None, target_bir_lowering=False)
    x_ext = nc.declare_dram_parameter("x", [D, TPC], bf16, isOutput=False)
    w_ext = nc.declare_dram_parameter("weight", [D, OUT], f32, isOutput=False)
    out_ext = nc.declare_dram_parameter("out", [TPC, OUT], f32, isOutput=True)

    with tile.TileContext(nc) as tc:
        with (
            tc.tile_pool(name="persist", bufs=1) as persist,
            tc.tile_pool(name="wf32", bufs=3) as wf32_pool,
            tc.tile_pool(name="sgn", bufs=4) as sgn_pool,
            tc.tile_pool(name="outp", bufs=3) as out_pool,
            tc.tile_pool(name="psum", bufs=8, space="PSUM") as psum_pool,
        ):
            xt = persist.tile([128, KT, TPC], bf16)      # x^T, all K-tiles
            wq = persist.tile([128, KT, OUT], bf16)      # quantized w^T (doubled)
            ones = persist.tile([128, 128], f32)
            partials = persist.tile([128, KT], f32)
            scale_sb = persist.tile([128, 1], f32)
            t_pos = persist.tile([128, 1], f32)
            t_neg = persist.tile([128, 1], f32)
            s_half = persist.tile([128, 1], f32)
            tot = persist.tile([128, 1], f32)

            nc.sync.dma_start(xt[:], x_ext[:].rearrange("(k p) m -> p k m", p=128))
            nc.vector.memset(ones[:], 1.0)

            # ---- pass 1 over w: |w| row-sums per K-tile ----
            for k in range(KT):
                wt = wf32_pool.tile([128, OUT], f32, tag="wf32")
                nc.sync.dma_start(wt[:], w_ext[k * 128 : (k + 1) * 128, :])
                nc.vector.tensor_reduce(
                    partials[:, k : k + 1],
                    wt[:],
                    axis=mybir.AxisListType.X,
                    op=Alu.add,
                    apply_absolute_value=True,
                )

            # ---- scale scalar, broadcast to all partitions via ones-matmul ----
            nc.vector.tensor_reduce(
                tot[:], partials[:], axis=mybir.AxisListType.X, op=Alu.add
            )
            pbc = psum_pool.tile([128, 512], f32, tag="psum", name="pbc")
            nc.tensor.matmul(pbc[:, 0:1], ones[:], tot[:], start=True, stop=True)
            nc.vector.tensor_scalar(
                scale_sb[:], pbc[:, 0:1], 1.0 / N_ELEM, EPS, Alu.mult, Alu.max
            )
            nc.vector.tensor_scalar(t_pos[:], scale_sb[:], 1.0 / 3.0, None, Alu.mult)
            nc.vector.tensor_scalar(t_neg[:], scale_sb[:], -1.0 / 3.0, None, Alu.mult)
            nc.vector.tensor_scalar(s_half[:], scale_sb[:], 0.5, None, Alu.mult)

            # ---- pass 2 over w: ternary quantization (doubled) ----
            for k in range(KT):
                wt = wf32_pool.tile([128, OUT], f32, tag="wf32")
                nc.sync.dma_start(wt[:], w_ext[k * 128 : (k + 1) * 128, :])
                s1 = sgn_pool.tile([128, OUT], bf16, tag="sgn")
                s2 = sgn_pool.tile([128, OUT], bf16, tag="sgn")
                nc.scalar.activation(s1[:], wt[:], Act.Sign, bias=t_pos[:, 0:1])
                nc.scalar.activation(s2[:], wt[:], Act.Sign, bias=t_neg[:, 0:1])
                nc.vector.tensor_tensor(wq[:, k, :], s1[:], s2[:], Alu.add)

            # ---- matmul: out[m,n] = sum_k xt[k,m].T @ wq[k,n] ----
            def do_mtile(ms):
                psums = [
                    psum_pool.tile([128, 512], f32, tag="psum", name=f"ps{i}")
                    for i in range(NT * len(ms))
                ]
                for k in range(KT):
                    for mi, m in enumerate(ms):
                        for n in range(NT):
                            nc.tensor.matmul(
                                psums[mi * NT + n][:],
                                xt[:, k, m * 128 : (m + 1) * 128],
                                wq[:, k, n * 512 : (n + 1) * 512],
                                start=(k == 0),
                                stop=(k == KT - 1),
                            )
                for mi, m in enumerate(ms):
                    ot = out_pool.tile([128, OUT], f32, tag="outp")
                    for n in range(NT):
                        nc.scalar.activation(
                            ot[:, n * 512 : (n + 1) * 512],
                            psums[mi * NT + n][:],
                            Act.Copy,
                            scale=s_half[:, 0:1],
                        )
                    nc.sync.dma_start(out_ext[m * 128 : (m + 1) * 128, :], ot[:])

            # first two m-tiles share the quant stream (k-outer, 8 psum banks)
            do_mtile([0, 1])
            for m in range(2, MT):
                do_mtile([m])

    nc.finalize()
    return nc


_NC_CACHE = None


def kernel(x, weight):
    global _NC_CACHE
    import ml_dtypes
    from concourse.bass_utils import run_bass_kernel_spmd

    x = np.asarray(x, dtype=np.float32).reshape(TOK, D)
    weight = np.asarray(weight, dtype=np.float32)
    wT = np.ascontiguousarray(weight.T)                      # [in, out] f32
    in_maps = []
    for i in range(N_CORES):
        shard = np.ascontiguousarray(x[i * TPC : (i + 1) * TPC].T)  # [in, tok]
        in_maps.append({"x": shard.astype(ml_dtypes.bfloat16), "weight": wT})

    if _NC_CACHE is None:
        _NC_CACHE = build_kernel()
    res = run_bass_kernel_spmd(_NC_CACHE, in_maps, core_ids=list(range(N_CORES)))
    outs = [res.results[i]["out"] for i in range(N_CORES)]
    return np.concatenate(outs, axis=0).reshape(B, S, OUT).astype(np.float32)
